# revision 11
# baseline (speedup 1.0000x reference)
"""Trainium2 Bass kernel for nn_Encoder_4724464025749 (tree-GRU encoder).

Strategy
--------
Pure data parallelism: batch B=4096 split across 8 NeuronCores (512 columns
each), 2 independent tree walks per core (256 columns each) interleaved for
scheduler overlap.  All tensors feature-major ([feature partitions, batch
columns]); hidden states never leave SBUF.

Precision: the attention normalization a = s/(s0+s1) makes the model chaotic
(f32-level noise already amplifies to ~3e-3 output rel err), so every
hidden-state matmul must be f32-exact.  Instead of true-f32 matmuls (4
cycles/row on PE) we use an exact 3-term f32r split at 1 cycle/row:
  W @ h = W_hi @ h_hi + W_hi @ h_lo + W_lo @ h_hi    (+ O(2^-23) dropped)
with W_hi = rne11(W) host-side and h_hi = trunc11(h) materialized on device
by one bitwise-AND (DVE/Pool) + one subtract.  That is 6 K-tile passes per
H x H matmul versus f32's effective 8 — a 25% PE reduction — plus the
attention-score matmul broadcasts its scalar across all 128 partitions for
free (cost scales only with N), removing the separate ones-broadcast matmul
and turning the a=s0/(s0+s1) arithmetic into plain full-partition DVE ops.
The x projections keep the exact hi/lo stacked-K trick from the baseline.
Split bit-ops are spread across DVE and the otherwise-idle Pool engine to
stay under the PE roofline.
"""

import numpy as np

DEPTH = 7
H = 256
I = 32
O = 128
B = 4096
NCORES = 8
P = 128
HT = H // P          # feature tiles per vector
KSP = 3 * I + 2      # split x contraction: xhi | 1 | xlo | 1 | xhi
CH = 4               # nodes per x/mask DMA chunk
NCOL = B // NCORES   # batch columns per core
SPLITS = 2           # independent tree walks per core
NSUB = NCOL // SPLITS
MASK11 = 0xFFFFF000  # keep sign+exp+11 mantissa bits


def _post_order(depth, block=4):
    """Post-order walk; subtrees rooted at `block` level are emitted
    internally in bottom-up level order (wider ready-set for the scheduler
    while keeping the DFS-bounded live set above the block level)."""
    order = []

    def rec(d, j):
        if d == block and depth - 1 > d:
            for dd in range(depth - 1, d - 1, -1):
                for jj in range(j << (dd - d), (j + 1) << (dd - d)):
                    order.append((dd, jj))
            return
        if d < depth - 1:
            rec(d + 1, 2 * j)
            rec(d + 1, 2 * j + 1)
        order.append((d, j))

    rec(0, 0)
    return order


def _gid(d, j):
    return 2 ** d - 1 + j


def _round11(x):
    """Round fp32 to 11 explicit mantissa bits (the f32r operand grid)."""
    x = np.ascontiguousarray(np.asarray(x, dtype=np.float32))
    b = x.view(np.uint32)
    r = ((b + np.uint32(0x800)) >> np.uint32(12)) << np.uint32(12)
    return r.view(np.float32)


_MODULE_CACHE = {}


def _build_module(depth=DEPTH, ncol=NCOL, use_bias=False, mode="f32",
                  num_devices=NCORES, splits=SPLITS, use_mask=True):
    key = (depth, ncol, use_bias, mode, num_devices, splits, use_mask)
    if key in _MODULE_CACHE:
        return _MODULE_CACHE[key]

    import concourse.mybir as mybir
    import concourse.tile as tile
    from concourse import bacc

    dt = mybir.dt
    ACT_F = mybir.ActivationFunctionType
    ALU = mybir.AluOpType
    f32 = dt.float32
    f32r = dt.float32r
    u32 = dt.uint32

    nodes = 2 ** depth - 1
    order = _post_order(depth)
    nsub = ncol // splits                        # columns per tree walk

    nc = bacc.Bacc("TRN2", num_devices=num_devices, debug=False)

    xT_d = nc.dram_tensor("xT", [KSP, nodes, ncol], f32r, kind="ExternalInput").ap()
    mb_d = nc.dram_tensor("maskb", [P, nodes, ncol], f32, kind="ExternalInput").ap()
    wi_d = nc.dram_tensor("wi", [KSP, 3 * H], f32r, kind="ExternalInput").ap()
    # hi/lo split H-weights, lhsT layout [P, HT, H]
    wh_names = ["whr", "whz", "whn", "wa", "wms0", "wms1"]
    wh_d = {}
    for nm in wh_names:
        wh_d[nm + "_hi"] = nc.dram_tensor(nm + "_hi", [P, HT, H], f32r,
                                          kind="ExternalInput").ap()
        wh_d[nm + "_lo"] = nc.dram_tensor(nm + "_lo", [P, HT, H], f32r,
                                          kind="ExternalInput").ap()
    wscb_d = nc.dram_tensor("wscb", [P, HT, P], f32, kind="ExternalInput").ap()
    wout_d = nc.dram_tensor("wout", [P, HT, 2 * O], f32r, kind="ExternalInput").ap()
    bias_d = nc.dram_tensor("biases", [P, 9], f32, kind="ExternalInput").ap()
    out_d = nc.dram_tensor("out", [2, P, ncol], f32, kind="ExternalOutput").ap()

    with tile.TileContext(nc) as tc:
        with tc.tile_pool(name="wpool", bufs=1) as wpool, \
             tc.tile_pool(name="xpool", bufs=2 * splits) as xpool, \
             tc.tile_pool(name="mpool", bufs=2 * splits) as mpool, \
             tc.tile_pool(name="hpool", bufs=13 * splits) as hpool, \
             tc.tile_pool(name="vpool", bufs=13 * splits) as vpool, \
             tc.tile_pool(name="clpool", bufs=6 * splits) as clpool, \
             tc.tile_pool(name="spool", bufs=4 * splits) as spool, \
             tc.tile_pool(name="opool", bufs=2) as opool, \
             tc.tile_pool(name="pp1", bufs=8, space="PSUM") as pp1:

            # ---- load weights once ----
            def wtile(dram, shape, dtype):
                t = wpool.tile(shape, dtype, tag=dram.name, name="w_" + dram.name)
                nc.sync.dma_start(out=t[:], in_=dram[:])
                return t

            wi_t = wtile(wi_d, [KSP, 3 * H], f32r)
            wh_t = {nm: wtile(wh_d[nm], [P, HT, H], f32r) for nm in wh_d}
            wscb_t = wtile(wscb_d, [P, HT, P], f32)
            wout_t = wtile(wout_d, [P, HT, 2 * O], f32r)
            bias_t = wpool.tile([P, 9], f32, tag="biases", name="biases_t")
            nc.sync.dma_start(out=bias_t[:], in_=bias_d[:])

            # chunked x / mask staging, per tree walk
            x_tiles = {}
            m_tiles = {}

            def get_chunk(w, t):
                c = t // CH
                if (w, c) not in x_tiles:
                    n0 = c * CH
                    n1 = min(n0 + CH, nodes)
                    c0, c1 = w * nsub, (w + 1) * nsub
                    xt = xpool.tile([KSP, CH, nsub], f32r, tag="xchunk",
                                    name="xchunk")
                    nc.sync.dma_start(out=xt[:, : n1 - n0, :],
                                      in_=xT_d[:, n0:n1, c0:c1])
                    if use_mask:
                        mt = mpool.tile([P, CH, nsub], f32, tag="mchunk",
                                        name="mchunk")
                        nc.sync.dma_start(out=mt[:, : n1 - n0, :],
                                          in_=mb_d[:, n0:n1, c0:c1])
                    else:
                        mt = None
                    x_tiles[(w, c)] = xt
                    m_tiles[(w, c)] = mt
                return x_tiles[(w, c)], m_tiles[(w, c)], t - c * CH

            def work(name="work"):
                return vpool.tile([P, HT, nsub], f32, tag="work", name=name)

            def split11(src, eng, tag):
                """hi = trunc11(src), lo = src - hi; both ops on one engine
                (same-engine ordering needs no semaphore)."""
                hi = clpool.tile([P, HT, nsub], f32, tag="hilo",
                                 name=tag + "_hi")
                lo = clpool.tile([P, HT, nsub], f32, tag="hilo",
                                 name=tag + "_lo")
                eng.tensor_scalar(
                    hi[:].bitcast(u32), src[:].bitcast(u32), MASK11, None,
                    ALU.bitwise_and)
                eng.tensor_sub(lo[:], src[:], hi[:])
                return hi, lo

            def mm_s3(ps_ap, nm, rhs_hi, rhs_lo, mt, start=True, stop=True):
                """ps_ap += W@rhs via the exact 3-term f32r split."""
                whi = wh_t[nm + "_hi"]
                wlo = wh_t[nm + "_lo"]
                terms = [(whi, rhs_hi), (whi, rhs_lo), (wlo, rhs_hi)]
                i = 0
                for wt, rhs in terms:
                    for kt in range(HT):
                        nc.tensor.matmul(
                            ps_ap,
                            lhsT=wt[:, kt, mt * P:(mt + 1) * P],
                            rhs=rhs[:, kt, :].bitcast(f32r),
                            start=(start and i == 0),
                            stop=(stop and i == 5),
                        )
                        i += 1

            def mm_x(ps_ap, mt, col0, xc, xi, start, stop):
                """ps_ap (+)= wi[:, col0+mt*P : col0+(mt+1)*P].T @ x."""
                nc.tensor.matmul(
                    ps_ap,
                    lhsT=wi_t[:, col0 + mt * P: col0 + (mt + 1) * P],
                    rhs=xc[:, xi, :],
                    start=start,
                    stop=stop,
                )

            def act(out_ap, in_ap, func, bias=0.0):
                nc.scalar.activation(out_ap, in_ap, func, bias=bias)

            def apply_mask(h_ap, val_ap, mc, xi):
                mbc = mc[:, xi:xi + 1, :].to_broadcast((P, HT, nsub))
                nc.vector.tensor_mul(h_ap, val_ap, mbc)

            def emit_leaf(w, t):
                xc, mc, xi = get_chunk(w, t)
                psz = pp1.tile([P, HT, nsub], f32, tag="ps1", name="pszl")
                psn = pp1.tile([P, HT, nsub], f32, tag="ps1", name="psnl")
                for mt in range(HT):
                    mm_x(psz[:, mt, :], mt, H, xc, xi, True, True)
                    mm_x(psn[:, mt, :], mt, 2 * H, xc, xi, True, True)
                z = work("zl")
                n = work("nl")
                act(z[:], psz[:], ACT_F.Sigmoid)
                act(n[:], psn[:], ACT_F.Tanh)
                t1 = work("t1l")
                nc.vector.tensor_mul(t1[:], z[:], n[:])
                h = hpool.tile([P, HT, nsub], f32, tag="h", name="h")
                if use_mask:
                    nc.vector.tensor_sub(t1[:], n[:], t1[:])
                    apply_mask(h[:], t1[:], mc, xi)
                else:
                    nc.vector.tensor_sub(h[:], n[:], t1[:])
                return h

            def emit_internal(w, t, d, hl, hr):
                xc, mc, xi = get_chunk(w, t)
                # children hi/lo splits: c0 on DVE, c1 on Pool (parallel)
                c0hi, c0lo = split11(hl, nc.vector, "c0")
                c1hi, c1lo = split11(hr, nc.gpsimd, "c1")
                diff = work("diff")
                nc.vector.tensor_sub(diff[:], hl[:], hr[:])

                # ---- attention chain first (long pole) ----
                # ms_k = tanh(Wms_k c_k); scores pre-broadcast across
                # partitions (lhsT columns all = w) as soon as each ms lands
                pss = pp1.tile([P, 2, nsub], f32, tag="ps1", name="pss")
                for k, (chi, clo) in ((0, (c0hi, c0lo)), (1, (c1hi, c1lo))):
                    psm = pp1.tile([P, HT, nsub], f32, tag="ps1", name="psm")
                    for mt in range(HT):
                        mm_s3(psm[:, mt, :], "wms%d" % k, chi, clo, mt)
                    ms = work("ms%d" % k)
                    if use_bias:
                        for mt in range(HT):
                            act(ms[:, mt, :], psm[:, mt, :], ACT_F.Tanh,
                                bias=bias_t[:, 2 * k + mt: 2 * k + mt + 1])
                    else:
                        act(ms[:], psm[:], ACT_F.Tanh)
                    for kt in range(HT):
                        nc.tensor.matmul(
                            pss[:, k, :],
                            lhsT=wscb_t[:, kt, :],
                            rhs=ms[:, kt, :],
                            start=(kt == 0),
                            stop=(kt == HT - 1),
                        )
                if use_bias:
                    nc.vector.tensor_scalar(pss[:], pss[:], bias_t[:, 8:9],
                                            None, ALU.add)
                den = spool.tile([P, 1, nsub], f32, tag="sm", name="den")
                rec = spool.tile([P, 1, nsub], f32, tag="sm", name="rec")
                a0 = spool.tile([P, 1, nsub], f32, tag="sm", name="a0")
                nc.vector.tensor_add(den[:, 0, :], pss[:, 0, :], pss[:, 1, :])
                nc.vector.reciprocal_approx_fast(rec[:, 0, :], den[:, 0, :])
                nc.vector.tensor_mul(a0[:, 0, :], pss[:, 0, :], rec[:, 0, :])

                # ---- g = c1 + a0*(c0-c1) ; cs = tanh(Wa g) ----
                g = work("g")
                a0b = a0[:, 0:1, :].to_broadcast((P, HT, nsub))
                nc.vector.tensor_mul(g[:], diff[:], a0b)
                nc.vector.tensor_add(g[:], g[:], hr[:])
                ghi, glo = split11(g, nc.gpsimd, "g")
                psc = pp1.tile([P, HT, nsub], f32, tag="ps1", name="psc")
                for mt in range(HT):
                    mm_s3(psc[:, mt, :], "wa", ghi, glo, mt)
                cs = work("cs")
                if use_bias:
                    for mt in range(HT):
                        act(cs[:, mt, :], psc[:, mt, :], ACT_F.Tanh,
                            bias=bias_t[:, 4 + mt: 5 + mt])
                else:
                    act(cs[:], psc[:], ACT_F.Tanh)
                cshi, cslo = split11(cs, nc.vector, "cs")
                psz = pp1.tile([P, HT, nsub], f32, tag="ps1", name="psz")
                for mt in range(HT):
                    mm_x(psz[:, mt, :], mt, H, xc, xi, True, False)
                    mm_s3(psz[:, mt, :], "whz", cshi, cslo, mt,
                          start=False, stop=True)
                z = work("z")
                act(z[:], psz[:], ACT_F.Sigmoid)

                # ---- r gates / s / n (short chain, fills engine gaps) ----
                rs = []
                for k, (chi, clo) in ((0, (c0hi, c0lo)), (1, (c1hi, c1lo))):
                    psr = pp1.tile([P, HT, nsub], f32, tag="ps1", name="psr")
                    for mt in range(HT):
                        mm_x(psr[:, mt, :], mt, 0, xc, xi, True, False)
                        mm_s3(psr[:, mt, :], "whr", chi, clo, mt,
                              start=False, stop=True)
                    r = work("r%d" % k)
                    act(r[:], psr[:], ACT_F.Sigmoid)
                    rs.append(r)
                s = work("s")
                t3 = work("t3")
                nc.vector.tensor_mul(s[:], rs[0][:], hl[:])
                nc.vector.tensor_mul(t3[:], rs[1][:], hr[:])
                nc.vector.tensor_add(s[:], s[:], t3[:])
                shi, slo = split11(s, nc.gpsimd, "s")
                psn = pp1.tile([P, HT, nsub], f32, tag="ps1", name="psn")
                for mt in range(HT):
                    mm_x(psn[:, mt, :], mt, 2 * H, xc, xi, True, False)
                    mm_s3(psn[:, mt, :], "whn", shi, slo, mt,
                          start=False, stop=True)
                n = work("n")
                act(n[:], psn[:], ACT_F.Tanh)

                # ---- h = (n + z*(cs - n)) * m ----
                t4 = work("t4")
                nc.vector.tensor_sub(t4[:], cs[:], n[:])
                nc.vector.tensor_mul(t4[:], z[:], t4[:])
                h = hpool.tile([P, HT, nsub], f32, tag="h", name="h")
                if use_mask:
                    nc.vector.tensor_add(t4[:], n[:], t4[:])
                    apply_mask(h[:], t4[:], mc, xi)
                else:
                    nc.vector.tensor_add(h[:], n[:], t4[:])
                return h

            # ---- walk the trees in interleaved, staggered post-order ----
            LAG = 16
            hmaps = [{} for _ in range(splits)]
            roots = [None] * splits

            def emit_one(w, ti):
                d, j = order[ti]
                hmap = hmaps[w]
                if d == depth - 1:
                    hmap[(d, j)] = emit_leaf(w, ti)
                else:
                    hl = hmap.pop((d + 1, 2 * j))
                    hr = hmap.pop((d + 1, 2 * j + 1))
                    hmap[(d, j)] = emit_internal(w, ti, d, hl, hr)

            n_nodes = len(order)
            for ti in range(n_nodes + LAG * (splits - 1)):
                for w in range(splits):
                    tw = ti - LAG * w
                    if 0 <= tw < n_nodes:
                        emit_one(w, tw)
            for w in range(splits):
                roots[w] = hmaps[w][(0, 0)]

            # ---- output heads ----
            for w in range(splits):
                root = roots[w]
                c0, c1 = w * nsub, (w + 1) * nsub
                pso = pp1.tile([P, 2, nsub], f32, tag="ps1", name="pso")
                for oi in range(2):
                    for kt in range(HT):
                        nc.tensor.matmul(
                            pso[:, oi, :],
                            lhsT=wout_t[:, kt, oi * O:(oi + 1) * O],
                            rhs=root[:, kt, :].bitcast(f32r),
                            start=(kt == 0),
                            stop=(kt == HT - 1),
                        )
                for oi in range(2):
                    ot = opool.tile([P, nsub], f32, tag="osb", name="osb")
                    if use_bias:
                        act(ot[:], pso[:, oi, :], ACT_F.Identity,
                            bias=bias_t[:, 6 + oi: 7 + oi])
                    else:
                        act(ot[:], pso[:, oi, :], ACT_F.Identity)
                    nc.sync.dma_start(out=out_d[oi, :, c0:c1], in_=ot[:])

    nc.compile()
    _MODULE_CACHE[key] = nc
    return nc


def _pack_weights(inputs, mode="f32"):
    """Host-side packing of weights into device lhsT layouts."""
    f = lambda k: np.asarray(inputs[k], dtype=np.float32)

    def lhsT_h(w):  # [H, H] torch-layout -> [P, HT, H]
        return np.ascontiguousarray(
            w.T.reshape(HT, P, w.shape[0]).transpose(1, 0, 2))

    wir_w, wiz_w, win_w = f("wir_w"), f("wiz_w"), f("win_w")
    br = f("wir_b") + f("whr_b")
    bz = f("wiz_b") + f("whz_b")
    bn = f("win_b") + f("whn_b")
    wcat = np.concatenate([wir_w, wiz_w, win_w], axis=0)      # [3H, I]
    bcat = np.concatenate([br, bz, bn])                       # [3H]
    # exact split-K layout: rows = xhi*Whi | 1*bhi | xlo*Whi | 1*blo | xhi*Wlo
    w_hi = _round11(wcat)
    w_lo = wcat - w_hi
    b_hi = _round11(bcat)
    b_lo = bcat - b_hi
    wi = np.concatenate([
        w_hi.T, b_hi[None, :], w_hi.T, b_lo[None, :], w_lo.T], axis=0)

    wms = f("wms_w")                                          # [2, H, H]
    packed = {"wi": np.ascontiguousarray(wi)}
    for nm, w in (("whr", f("whr_w")), ("whz", f("whz_w")),
                  ("whn", f("whn_w")), ("wa", f("wa_w")),
                  ("wms0", wms[0]), ("wms1", wms[1])):
        wl = lhsT_h(w)
        hi = _round11(wl)
        packed[nm + "_hi"] = hi
        packed[nm + "_lo"] = np.ascontiguousarray(wl - hi)

    wsc = f("w_w").reshape(H).reshape(HT, P).transpose(1, 0)  # [P, HT]
    packed["wscb"] = np.ascontiguousarray(
        np.broadcast_to(wsc[:, :, None], (P, HT, P)).astype(np.float32))
    packed["wout"] = lhsT_h(np.concatenate([f("mu_w"), f("lv_w")], axis=0))

    biases = np.zeros((P, 9), dtype=np.float32)
    wms_b = f("wms_b")                                        # [2, H]
    for k in range(2):
        for mt in range(HT):
            biases[:, 2 * k + mt] = wms_b[k, mt * P:(mt + 1) * P]
    wa_b = f("wa_b")
    for mt in range(HT):
        biases[:, 4 + mt] = wa_b[mt * P:(mt + 1) * P]
    biases[:, 6] = f("mu_b")
    biases[:, 7] = f("lv_b")
    biases[:, 8] = float(np.asarray(inputs["w_b"]).reshape(-1)[0])
    packed["biases"] = biases

    use_bias = any(
        float(np.abs(np.asarray(inputs[k])).max()) != 0.0
        for k in ("wms_b", "wa_b", "w_b", "mu_b", "lv_b")
    )
    return packed, use_bias


def _pack_percore(targets, mask, mode="f32", depth=DEPTH, ncol=NCOL,
                  ncores=NCORES):
    order = _post_order(depth)
    perm = np.array([_gid(d, j) for (d, j) in order])
    nodes = len(order)
    bsz = targets.shape[1]

    tg = np.asarray(targets, dtype=np.float32)[:, :, 0, :]    # [nodes, B, I]
    xall = tg.transpose(2, 0, 1)[:, perm, :]                  # [I, nodes, B]
    x_hi = _round11(xall)
    x_lo = xall - x_hi
    ones = np.ones((1, nodes, bsz), np.float32)
    xaug = np.concatenate([x_hi, ones, x_lo, ones, x_hi], axis=0)  # [KSP,...]
    mpost = np.asarray(mask, dtype=np.float32)[perm]          # [nodes, B]

    per_core = []
    for c in range(ncores):
        cols = slice(c * ncol, (c + 1) * ncol)
        xc = np.ascontiguousarray(xaug[:, :, cols])
        mc = np.ascontiguousarray(
            np.broadcast_to(mpost[None, :, cols], (P, nodes, ncol)))
        per_core.append({"xT": xc, "maskb": mc})
    return per_core


def kernel(**inputs):
    import sys
    try:
        import concourse.bass  # noqa: F401
    except ImportError:
        sys.path.insert(0, "/opt/trn_rl_repo")

    try:
        import antenv.axon_hooks  # noqa: F401
    except ImportError:
        # absent in trimmed containers; run_bass_kernel_spmd imports it
        # unconditionally when BASS_TRACE is set — stub the no-hook path
        import types
        _m = types.ModuleType("antenv.axon_hooks")
        _m.get_axon_ntff_profile_hook = lambda: None
        sys.modules["antenv.axon_hooks"] = _m

    from concourse import bass_utils

    packed, use_bias = _pack_weights(inputs)
    use_mask = bool(np.any(np.asarray(inputs["mask"]) != 1.0))
    nc = _build_module(use_bias=use_bias, use_mask=use_mask)
    per_core = _pack_percore(inputs["targets"], inputs["mask"])

    in_maps = [{**pc, **packed} for pc in per_core]
    res = bass_utils.run_bass_kernel_spmd(
        nc, in_maps, core_ids=list(range(NCORES)))

    mu = np.empty((B, 1, O), dtype=np.float32)
    lv = np.empty((B, 1, O), dtype=np.float32)
    for c in range(NCORES):
        out = res.results[c]["out"]                          # [2, P, ncol]
        cols = slice(c * NCOL, (c + 1) * NCOL)
        mu[cols, 0, :] = out[0].T
        lv[cols, 0, :] = out[1].T
    return mu, lv
